# revision 19
# baseline (speedup 1.0000x reference)
"""Trainium2 Bass kernel for nn_CategoryAlign_Module (pooling / cross Pearson).

Math (see reference):
  for each stream s in {1,2}:
    vec_b[k,c]  = sum_p preds[b,k,p] * feats[b,c,p] / sum_p preds[b,k,p]
    ctx_b[k,c]  = vec_b[k,c] / max(||vec_b[:,c]||_2, 1e-12)      (norm over K)
    ctx[k,c]    = mean_b ctx_b[k,c]
  out = pearson(ctx1, ctx2)   (center+normalize rows over C, then ctx1 @ ctx2^T)

Distribution: data-parallel over the batch dim, one batch element per
NeuronCore (B=8, 8 cores).  Each core computes its local normalized
contexts, the tiny [19,257] payload is AllReduce-summed across the 8
cores (Pearson is invariant to the 1/B scale, so the mean's division is
skipped), and every core redundantly computes the replicated [19,19]
correlation.

Per-core pipeline (bf16 compute / fp32 accumulate):
  - both preds and feats are host-relayouted and host-cast to bf16, so
    the device only moves half the bytes and never transposes:
      preds -> [128, 128*19]  (chunk h: P^T[h*128:(h+1)*128, :19])
      feats -> [128, 128*257] (chunk h: [w, c] block with a fused ones
               column, so psum[:, 256] accumulates the mask sums)
  - bulk DMA is split across the two HWDGE queues (sync + scalar
    engines), 16-chunk segments, so both queues stream concurrently
  - 128 accumulating matmuls per stream produce [19, 257] in PSUM
  - stream 0's AllReduce launches at the halfway point and hides under
    stream 1's DMA; only stream 1's AllReduce + the tiny Pearson
    epilogue are exposed at the tail
"""

import sys

sys.path.insert(0, "/opt/trn_rl_repo")

import numpy as np

import concourse.bass as bass  # noqa: F401  (import order matters)
import concourse.bacc as bacc
import concourse.tile as tile
import concourse.mybir as mybir
from concourse import bass_utils, bass2jax  # noqa: F401

B, K, C, H, W = 8, 19, 256, 128, 128
P = H * W            # 16384 spatial positions
NCHUNK = P // 128    # 128 contraction chunks
CCW = C + 1          # channels + fused ones column (mask sums)
# DMA segment sizes in 128-position chunks: big segments for the bulk
# of the window, small ones at the end so the last arrival gates as
# little compute as possible
SEG_SIZES = (32, 32, 32, 16, 8, 8)
N_CORES = 8

F32 = mybir.dt.float32
BF16 = mybir.dt.bfloat16


def build_body(nc, tc, pret_d, ftr_d, identf_d, out_d, n_cores):
    """Emit the per-core program.

    pret_d: 2 DRAM APs [128, NCHUNK*K] bf16 (preds, spatial-major relayout)
    ftr_d:  2 DRAM APs [128, NCHUNK*CCW] bf16 (feats, spatial-major
            relayout + ones column)
    identf_d: [K, K] fp32 identity (for the tiny Pearson transposes)
    out_d:  [K, K] fp32 output
    """
    add = mybir.AluOpType.add
    mult = mybir.AluOpType.mult

    with tc.tile_pool(name="persist", bufs=1) as PP, \
         tc.tile_pool(name="ft", bufs=1) as FTP, \
         tc.tile_pool(name="acc", bufs=1, space="PSUM") as PA, \
         tc.tile_pool(name="tail", bufs=1, space="PSUM") as TLP, \
         tc.tile_pool(name="dram", bufs=1, space="DRAM") as DP:

        ones19 = PP.tile([K, 1], F32, name="ones19")
        nc.vector.memset(ones19[:], 1.0)
        onesrow = PP.tile([1, K], F32, name="onesrow")
        nc.vector.memset(onesrow[:], 1.0)

        # --- bulk DMA: both streams across the two HWDGE queues,
        # interleaved so stream 0 lands in the first half of the DMA
        # window.  The tiny constant loads and the warmup-collective
        # staging ride the same rings right after the first segments. ---
        qs = [nc.sync, nc.scalar]
        id_f = PP.tile([32 + K, K], F32, name="id_f")
        dw_in = DP.tile([1, 1], F32, name="dw_in")
        dw_out = DP.tile([1, 1], F32, name="dw_out")
        PT = []
        for s in (0, 1):
            pt = PP.tile([128, NCHUNK * K], BF16, name=f"PT{s}")
            qs[s].dma_start(pt[:], pret_d[s][:])
            PT.append(pt)
        fseg = [[], []]   # list of (tile, chunk_base, n_chunks)
        for s in (0, 1):
            base = 0
            for si, nch in enumerate(SEG_SIZES):
                t_ = FTP.tile([128, nch * CCW], BF16, name=f"fs{s}{si}")
                # middle segments ride SWDGE as a third queue; the two
                # HWDGE engines then never exceed the 4-deep issue queue
                q = nc.gpsimd if si in (2, 3) else qs[(si + s) % 2]
                q.dma_start(
                    t_[:], ftr_d[s][:, base * CCW:(base + nch) * CCW])
                fseg[s].append((t_, base, nch))
                base += nch
                if s == 0 and si == 0:
                    nc.sync.dma_start(dw_in[:], ones19[0:1, 0:1])
                    nc.sync.dma_start(id_f[0:K, :], identf_d[:])
                elif s == 0 and si == 1:
                    nc.scalar.dma_start(id_f[32:32 + K, :], identf_d[:])

        # --- warmup AllReduce: absorbs the collective stack's one-time
        # trigger/mesh startup (~11us) while the bulk DMA streams; only
        # the gpsimd engine blocks on it. ---
        prev_cc = nc.gpsimd.collective_compute(
            "AllReduce", mybir.AluOpType.add,
            replica_groups=[list(range(n_cores))],
            ins=[dw_in.opt()], outs=[dw_out.opt()])

        # --- dummy activations preload the scalar engine's Square /
        # Rsqrt / Copy tables so the epilogues don't stall on
        # ACT_TABLE_LOAD (hidden under the bulk DMA window) ---
        wa = PP.tile([1, 1], F32, name="wa")
        wb = PP.tile([1, 1], F32, name="wb")
        nc.scalar.square(wa[:], ones19[0:1, 0:1])
        nc.scalar.sqrt(wb[:], ones19[0:1, 0:1])
        nc.scalar.activation(wa[:], ones19[0:1, 0:1],
                             mybir.ActivationFunctionType.Copy,
                             scale=0.5, accum_out=wb[:])

        # --- per-stream accumulators + shared payload tile (side s at
        # partition base 32*s: engines need operand bases in {0,32,64}) ---
        psum_vec = [PA.tile([K, CCW], F32, name=f"pvec{s}") for s in (0, 1)]
        cc_in = PP.tile([32 + K, CCW], F32, name="cc_in")
        cs = PP.tile([32 + K, CCW], F32, name="csum")
        # pad rows 19..31 so the merged tail ops stay finite there
        nc.vector.memset(cs[0:32, 0:C], 1.0)
        nc.vector.memset(cs[0:32, C:CCW], 0.0)

        for s in (0, 1):
            # ---- contraction: 128 accumulating matmuls ----
            for t_, base, nch in fseg[s]:
                for t in range(nch):
                    i = base + t
                    nc.tensor.matmul(
                        psum_vec[s][:],
                        lhsT=PT[s][:, i * K:(i + 1) * K],
                        rhs=t_[:, t * CCW:(t + 1) * CCW],
                        start=(i == 0), stop=(i == NCHUNK - 1))

            # ---- stream epilogue (stream 0's is fully hidden; only
            # stream 1's chain precedes the collective) ----
            # vec = psum[:, :C] / masksum;  ctx = vec / ||vec||_col
            rows = cc_in[32 * s:32 * s + K, :]
            recip = PP.tile([K, 1], F32, name="recip")
            nc.vector.reciprocal(recip[:], psum_vec[s][:, C:C + 1])
            rsq = PP.tile([K, 1], F32, name="rsq")
            nc.vector.tensor_mul(rsq[:], recip[:], recip[:])
            sq = PP.tile([K, C], F32, name="sq")
            nc.scalar.square(sq[:], psum_vec[s][:, 0:C])
            # col sums over K of (psum^2 * recip^2) = ||vec||^2
            pn = TLP.tile([1, C], F32, name="pn", tag="tlp")
            nc.tensor.matmul(pn[:], lhsT=rsq[:], rhs=sq[:],
                             start=True, stop=True)
            # reference clamps the norm at 1e-12; the norm here is
            # O(1e-2) for non-degenerate input, so the clamp is a no-op.
            nsb = PP.tile([1, C], F32, name="nsb")
            nc.scalar.sqrt(nsb[:], pn[:])
            rn = PP.tile([1, C], F32, name="rn")
            nc.vector.reciprocal(rn[:], nsb[:])
            # broadcast 1/norm to the K partitions (rank-1 matmul)
            bc = TLP.tile([K, C], F32, name="bc", tag="tlp")
            nc.tensor.matmul(bc[:], lhsT=onesrow[:], rhs=rn[:],
                             start=True, stop=True)
            bcs = PP.tile([K, C], F32, name="bcs")
            nc.scalar.copy(bcs[:], bc[:])
            # fused: ctx = (psum * recip) * bc, with the row-SUM shipped
            # in the payload's last column (the tail divides it by C)
            nc.vector.scalar_tensor_tensor(
                rows[:, 0:C], psum_vec[s][:, 0:C], recip[:], bcs[:],
                op0=mult, op1=mult, accum_out=rows[:, C:C + 1])

        # ---- ONE AllReduce of both payloads [38, 257]; staging and
        # return halves ride the two HWDGE rings in parallel (the bulk
        # has drained by then), gpsimd only triggers the collective ----
        b_in = DP.tile([2 * K, CCW], F32, name="b_in")
        b_out = DP.tile([2 * K, CCW], F32, name="b_out")
        for s in (0, 1):
            qs[s].dma_start(b_in[s * K:(s + 1) * K, :],
                            cc_in[32 * s:32 * s + K, :])
        cc = nc.gpsimd.collective_compute(
            "AllReduce", mybir.AluOpType.add,
            replica_groups=[list(range(n_cores))],
            ins=[b_in.opt()], outs=[b_out.opt()])
        bass._add_dep_helper(cc.ins, prev_cc.ins, sync=False,
                             reason="collectives in program order")
        for s in (0, 1):
            qs[s].dma_start(cs[32 * s:32 * s + K, :],
                            b_out[s * K:(s + 1) * K, :])

        # ---- Pearson prep, both sides at once ([51] partitions).
        # ms holds row SUMS; mean m = ms/C, and
        # sum_c (X-m)^2 = sum_c X^2 - ms^2/C  (no explicit centering) ----
        X = cs[:, 0:C]
        ms = cs[:, C:C + 1]
        xsq = PP.tile([32 + K, C], F32, name="xsq")
        ssx = PP.tile([32 + K, 1], F32, name="ssx")
        nc.scalar.activation(xsq[:], X,
                             mybir.ActivationFunctionType.Square,
                             accum_out=ssx[:])
        msc = PP.tile([32 + K, 1], F32, name="msc")
        nc.vector.tensor_scalar_mul(msc[:], ms, 1.0 / C)
        w = PP.tile([32 + K, 1], F32, name="w")
        nc.vector.tensor_scalar(w[:], ms, ms, 1.0 / C,
                                op0=mult, op1=mult)
        ss = PP.tile([32 + K, 1], F32, name="ss")
        nc.vector.tensor_sub(ss[:], ssx[:], w[:])
        sd = PP.tile([32 + K, 1], F32, name="sd")
        nc.scalar.sqrt(sd[:], ss[:])
        ri = PP.tile([32 + K, 1], F32, name="ri")
        nc.vector.reciprocal(ri[:], sd[:])
        # split per side so the transpose's lhsT sits at partition base
        # 0 / 32 (PE requires base in {0, 32, 64})
        xn = PP.tile([32 + K, C], F32, name="xn")
        for s in (0, 1):
            nc.vector.tensor_scalar(xn[32 * s:32 * s + K, :],
                                    X[32 * s:32 * s + K, :],
                                    msc[32 * s:32 * s + K, :],
                                    ri[32 * s:32 * s + K, :],
                                    op0=mybir.AluOpType.subtract,
                                    op1=mybir.AluOpType.mult)
        # transpose both sides' [K, C] -> [C, K], 4 blocks of 128
        tps = TLP.tile([128, 4 * K], F32, name="tps", tag="tlp")
        for j in range(4):
            s, h = j // 2, j % 2
            nc.tensor.matmul(
                tps[:, j * K:(j + 1) * K],
                lhsT=xn[32 * s:32 * s + K, h * 128:(h + 1) * 128],
                rhs=id_f[32 * s:32 * s + K, :],
                is_transpose=True,
                start=(j == 0), stop=(j == 3))
        nT = PP.tile([128, 4 * K], F32, name="nT")
        nc.vector.tensor_copy(nT[:], tps[:])

        # ---- final correlation ----
        po = TLP.tile([K, K], F32, name="po", tag="tlp")
        for h in (0, 1):
            nc.tensor.matmul(po[:],
                             lhsT=nT[:, h * K:(h + 1) * K],
                             rhs=nT[:, (2 + h) * K:(3 + h) * K],
                             start=(h == 0), stop=(h == 1))
        osb = PP.tile([K, K], F32, name="osb")
        nc.vector.tensor_copy(osb[:], po[:])
        nc.sync.dma_start(out_d[:], osb[:])


def build(n_cores=N_CORES):
    nc = bacc.Bacc("TRN2", target_bir_lowering=False, debug=False,
                   enable_asserts=False, num_devices=n_cores)
    pret_d = [nc.dram_tensor(f"pret{s}", [128, NCHUNK * K], BF16,
                             kind="ExternalInput").ap() for s in (1, 2)]
    ftr_d = [nc.dram_tensor(f"ftr{s}", [128, NCHUNK * CCW], BF16,
                            kind="ExternalInput").ap() for s in (1, 2)]
    identf_d = nc.dram_tensor("identf", [K, K], F32, kind="ExternalInput").ap()
    out_d = nc.dram_tensor("out", [K, K], F32, kind="ExternalOutput").ap()
    with tile.TileContext(nc) as tc:
        build_body(nc, tc, pret_d, ftr_d, identf_d, out_d, n_cores)
    nc.compile()
    return nc


_NC_CACHE = {}


def _get_nc():
    if "nc" not in _NC_CACHE:
        _NC_CACHE["nc"] = build(N_CORES)
    return _NC_CACHE["nc"]


class Runner:
    """Executes the compiled Bass program on the first `n_cores` jax
    devices via shard_map, with inputs pre-staged on the devices (the
    analog of the native path's input pre-load in run_neff) so all
    cores start the NEFF near-simultaneously."""

    def __init__(self, nc, n_cores):
        import jax
        from jax.experimental.shard_map import shard_map
        from jax.sharding import Mesh, PartitionSpec, NamedSharding

        bass2jax.install_neuronx_cc_hook()
        self.jax = jax
        self.nc = nc
        self.n_cores = n_cores
        assert nc.dbg_addr is None
        partition_name = (nc.partition_id_tensor.name
                          if nc.partition_id_tensor else None)
        in_names, out_names, out_avals = [], [], []
        for alloc in nc.m.functions[0].allocations:
            if not isinstance(alloc, mybir.MemoryLocationSet):
                continue
            name = alloc.memorylocations[0].name
            if alloc.kind == "ExternalInput":
                if name != partition_name:
                    in_names.append(name)
            elif alloc.kind == "ExternalOutput":
                shape = tuple(alloc.tensor_shape)
                dtype = mybir.dt.np(alloc.dtype)
                out_names.append(name)
                out_avals.append(jax.core.ShapedArray(shape, dtype))
        self.param_names = list(in_names)
        n_params = len(in_names)
        full_in_names = list(in_names) + list(out_names)
        if partition_name is not None:
            full_in_names.append(partition_name)
        full_in_names = tuple(full_in_names)
        donate = tuple(range(n_params, n_params + len(out_names)))
        self.out_names = out_names
        self.out_avals = out_avals

        def _body(*args):
            operands = list(args)
            if partition_name is not None:
                operands.append(bass2jax.partition_id_tensor())
            outs = bass2jax._bass_exec_p.bind(
                *operands,
                out_avals=tuple(out_avals),
                in_names=full_in_names,
                out_names=tuple(out_names),
                lowering_input_output_aliases=(),
                sim_require_finite=True,
                sim_require_nnan=True,
                nc=nc,
            )
            return tuple(outs)

        devices = jax.devices()[:n_cores]
        assert len(devices) == n_cores
        self.mesh = Mesh(np.asarray(devices), ("core",))
        in_specs = (PartitionSpec("core"),) * (n_params + len(out_names))
        out_specs = (PartitionSpec("core"),) * len(out_names)
        self.fn = jax.jit(
            shard_map(_body, mesh=self.mesh, in_specs=in_specs,
                      out_specs=out_specs, check_rep=False),
            donate_argnums=donate, keep_unused=True)
        self.sharding = NamedSharding(self.mesh, PartitionSpec("core"))

    def put(self, in_maps):
        concat = [
            np.concatenate([np.asarray(in_maps[c][n])
                            for c in range(self.n_cores)], axis=0)
            for n in self.param_names
        ]
        arrs = [self.jax.device_put(a, self.sharding) for a in concat]
        self.jax.block_until_ready(arrs)
        return arrs

    def zeros(self):
        zs = [self.jax.device_put(
            np.zeros((self.n_cores * a.shape[0], *a.shape[1:]), a.dtype),
            self.sharding) for a in self.out_avals]
        self.jax.block_until_ready(zs)
        return zs

    def exec(self, dev_in):
        outs = self.fn(*dev_in, *self.zeros())
        self.jax.block_until_ready(outs)
        return {
            name: np.asarray(outs[i]).reshape(
                self.n_cores, *self.out_avals[i].shape)
            for i, name in enumerate(self.out_names)
        }


def _get_runner():
    if "runner" not in _NC_CACHE:
        _NC_CACHE["runner"] = Runner(_get_nc(), N_CORES)
    return _NC_CACHE["runner"]


def make_in_maps(preds1, feats1, preds2, feats2):
    import ml_dtypes
    bf16 = ml_dtypes.bfloat16
    identf = np.eye(K, dtype=np.float32)
    per_stream = {}
    for s, (preds, feats) in enumerate(
            ((preds1, feats1), (preds2, feats2)), start=1):
        # preds [B,K,H,W] -> [B, W(v), H(u), K] -> [B, 128, 128*19]:
        # chunk u's columns are P^T[u*128:(u+1)*128, :19] with the
        # spatial index on partitions
        pr = np.ascontiguousarray(
            preds.astype(bf16).transpose(0, 3, 2, 1)
        ).reshape(B, 128, NCHUNK * K)
        # feats [B,C,H,W] -> [B, W, H, C (+ ones)] -> [B, 128, 128*257]:
        # chunk u is the [w, c] block at h=u, matching pret's chunking;
        # the fused ones column makes psum[:, 256] the mask sums
        ft = np.empty((B, W, H, CCW), dtype=bf16)
        ft[..., :C] = feats.astype(bf16).transpose(0, 3, 2, 1)
        ft[..., C] = 1.0
        per_stream[s] = (pr, ft.reshape(B, 128, NCHUNK * CCW))
    in_maps = []
    for b in range(B):
        in_maps.append({
            "pret1": per_stream[1][0][b],
            "pret2": per_stream[2][0][b],
            "ftr1": per_stream[1][1][b],
            "ftr2": per_stream[2][1][b],
            "identf": identf,
        })
    return in_maps


def kernel(preds1, feats1, preds2, feats2):
    runner = _get_runner()
    in_maps = make_in_maps(preds1, feats1, preds2, feats2)
    dev_in = runner.put(in_maps)
    outs = runner.exec(dev_in)
    return np.asarray(outs["out"][0], dtype=np.float32)


# revision 21
# speedup vs baseline: 1.0401x; 1.0401x over previous
"""Trainium2 Bass kernel for nn_CategoryAlign_Module (pooling / cross Pearson).

Math (see reference):
  for each stream s in {1,2}:
    vec_b[k,c]  = sum_p preds[b,k,p] * feats[b,c,p] / sum_p preds[b,k,p]
    ctx_b[k,c]  = vec_b[k,c] / max(||vec_b[:,c]||_2, 1e-12)      (norm over K)
    ctx[k,c]    = mean_b ctx_b[k,c]
  out = pearson(ctx1, ctx2)   (center+normalize rows over C, then ctx1 @ ctx2^T)

Distribution: data-parallel over the batch dim, one batch element per
NeuronCore (B=8, 8 cores).  Each core computes its local normalized
contexts, the tiny [19,257] payload is AllReduce-summed across the 8
cores (Pearson is invariant to the 1/B scale, so the mean's division is
skipped), and every core redundantly computes the replicated [19,19]
correlation.

Per-core pipeline (bf16 compute / fp32 accumulate):
  - both preds and feats are host-relayouted and host-cast to bf16, so
    the device only moves half the bytes and never transposes:
      preds -> [128, 128*19]  (chunk h: P^T[h*128:(h+1)*128, :19])
      feats -> [128, 128*257] (chunk h: [w, c] block with a fused ones
               column, so psum[:, 256] accumulates the mask sums)
  - bulk DMA is split across the two HWDGE queues (sync + scalar
    engines), 16-chunk segments, so both queues stream concurrently
  - 128 accumulating matmuls per stream produce [19, 257] in PSUM
  - stream 0's AllReduce launches at the halfway point and hides under
    stream 1's DMA; only stream 1's AllReduce + the tiny Pearson
    epilogue are exposed at the tail
"""

import sys

sys.path.insert(0, "/opt/trn_rl_repo")

import numpy as np

import concourse.bass as bass  # noqa: F401  (import order matters)
import concourse.bacc as bacc
import concourse.tile as tile
import concourse.mybir as mybir
from concourse import bass_utils, bass2jax  # noqa: F401

B, K, C, H, W = 8, 19, 256, 128, 128
P = H * W            # 16384 spatial positions
NCHUNK = P // 128    # 128 contraction chunks
CCW = C + 1          # channels + fused ones column (mask sums)
# DMA segment sizes in 128-position chunks: big segments for the bulk
# of the window, small ones at the end so the last arrival gates as
# little compute as possible
SEG_SIZES = (32, 32, 32, 16, 8, 8)
N_CORES = 8

F32 = mybir.dt.float32
BF16 = mybir.dt.bfloat16


def build_body(nc, tc, pret_d, ftr_d, identf_d, out_d, n_cores):
    """Emit the per-core program.

    pret_d: 2 DRAM APs [128, NCHUNK*K] bf16 (preds, spatial-major relayout)
    ftr_d:  2 DRAM APs [128, NCHUNK*CCW] bf16 (feats, spatial-major
            relayout + ones column)
    identf_d: [K, K] fp32 identity (for the tiny Pearson transposes)
    out_d:  [K, K] fp32 output
    """
    add = mybir.AluOpType.add
    mult = mybir.AluOpType.mult

    with tc.tile_pool(name="persist", bufs=1) as PP, \
         tc.tile_pool(name="ft", bufs=1) as FTP, \
         tc.tile_pool(name="acc", bufs=1, space="PSUM") as PA, \
         tc.tile_pool(name="tail", bufs=1, space="PSUM") as TLP, \
         tc.tile_pool(name="dram", bufs=1, space="DRAM") as DP:

        ones19 = PP.tile([K, 1], F32, name="ones19")
        nc.vector.memset(ones19[:], 1.0)
        onesrow = PP.tile([1, K], F32, name="onesrow")
        nc.vector.memset(onesrow[:], 1.0)

        # --- bulk DMA: both streams across the two HWDGE queues,
        # interleaved so stream 0 lands in the first half of the DMA
        # window.  The tiny constant loads and the warmup-collective
        # staging ride the same rings right after the first segments. ---
        qs = [nc.sync, nc.scalar]
        id_f = PP.tile([32 + K, K], F32, name="id_f")
        dw_in = DP.tile([1, 1], F32, name="dw_in")
        dw_out = DP.tile([1, 1], F32, name="dw_out")
        PT = []
        for s in (0, 1):
            pt = PP.tile([128, NCHUNK * K], BF16, name=f"PT{s}")
            qs[s].dma_start(pt[:], pret_d[s][:])
            PT.append(pt)
        fseg = [[], []]   # list of (tile, chunk_base, n_chunks)
        for s in (0, 1):
            base = 0
            for si, nch in enumerate(SEG_SIZES):
                t_ = FTP.tile([128, nch * CCW], BF16, name=f"fs{s}{si}")
                qs[(si + s) % 2].dma_start(
                    t_[:], ftr_d[s][:, base * CCW:(base + nch) * CCW])
                fseg[s].append((t_, base, nch))
                base += nch
                if s == 0 and si == 0:
                    nc.sync.dma_start(dw_in[:], ones19[0:1, 0:1])
                    nc.sync.dma_start(id_f[0:K, :], identf_d[:])
                elif s == 0 and si == 1:
                    nc.scalar.dma_start(id_f[32:32 + K, :], identf_d[:])

        # --- warmup AllReduce: absorbs the collective stack's one-time
        # trigger/mesh startup (~11us) while the bulk DMA streams; only
        # the gpsimd engine blocks on it. ---
        prev_cc = nc.gpsimd.collective_compute(
            "AllReduce", mybir.AluOpType.add,
            replica_groups=[list(range(n_cores))],
            ins=[dw_in.opt()], outs=[dw_out.opt()])

        # --- dummy activations preload the scalar engine's Square /
        # Rsqrt / Copy tables so the epilogues don't stall on
        # ACT_TABLE_LOAD (hidden under the bulk DMA window) ---
        wa = PP.tile([1, 1], F32, name="wa")
        wb = PP.tile([1, 1], F32, name="wb")
        nc.scalar.square(wa[:], ones19[0:1, 0:1])
        nc.scalar.sqrt(wb[:], ones19[0:1, 0:1])
        nc.scalar.activation(wa[:], ones19[0:1, 0:1],
                             mybir.ActivationFunctionType.Copy,
                             scale=0.5, accum_out=wb[:])

        # --- per-stream accumulators + shared payload tile (side s at
        # partition base 32*s: engines need operand bases in {0,32,64}) ---
        psum_vec = [PA.tile([K, CCW], F32, name=f"pvec{s}") for s in (0, 1)]
        cc_in = PP.tile([32 + K, CCW], F32, name="cc_in")
        cs = PP.tile([32 + K, CCW], F32, name="csum")
        # pad rows 19..31 so the merged tail ops stay finite there
        nc.vector.memset(cs[0:32, 0:C], 1.0)
        nc.vector.memset(cs[0:32, C:CCW], 0.0)

        for s in (0, 1):
            # ---- contraction: 128 accumulating matmuls ----
            for t_, base, nch in fseg[s]:
                for t in range(nch):
                    i = base + t
                    nc.tensor.matmul(
                        psum_vec[s][:],
                        lhsT=PT[s][:, i * K:(i + 1) * K],
                        rhs=t_[:, t * CCW:(t + 1) * CCW],
                        start=(i == 0), stop=(i == NCHUNK - 1))

            # ---- stream epilogue (stream 0's is fully hidden; only
            # stream 1's chain precedes the collective) ----
            # vec = psum[:, :C] / masksum;  ctx = vec / ||vec||_col
            rows = cc_in[32 * s:32 * s + K, :]
            recip = PP.tile([K, 1], F32, name="recip")
            nc.vector.reciprocal(recip[:], psum_vec[s][:, C:C + 1])
            rsq = PP.tile([K, 1], F32, name="rsq")
            nc.vector.tensor_mul(rsq[:], recip[:], recip[:])
            sq = PP.tile([K, C], F32, name="sq")
            nc.scalar.square(sq[:], psum_vec[s][:, 0:C])
            # col sums over K of (psum^2 * recip^2) = ||vec||^2
            pn = TLP.tile([1, C], F32, name="pn", tag="tlp")
            nc.tensor.matmul(pn[:], lhsT=rsq[:], rhs=sq[:],
                             start=True, stop=True)
            # reference clamps the norm at 1e-12; the norm here is
            # O(1e-2) for non-degenerate input, so the clamp is a no-op.
            nsb = PP.tile([1, C], F32, name="nsb")
            nc.scalar.sqrt(nsb[:], pn[:])
            rn = PP.tile([1, C], F32, name="rn")
            nc.vector.reciprocal(rn[:], nsb[:])
            # broadcast 1/norm to the K partitions (rank-1 matmul)
            bc = TLP.tile([K, C], F32, name="bc", tag="tlp")
            nc.tensor.matmul(bc[:], lhsT=onesrow[:], rhs=rn[:],
                             start=True, stop=True)
            bcs = PP.tile([K, C], F32, name="bcs")
            nc.scalar.copy(bcs[:], bc[:])
            # fused: ctx = (psum * recip) * bc, with the row-SUM shipped
            # in the payload's last column (the tail divides it by C)
            nc.vector.scalar_tensor_tensor(
                rows[:, 0:C], psum_vec[s][:, 0:C], recip[:], bcs[:],
                op0=mult, op1=mult, accum_out=rows[:, C:C + 1])

        # ---- ONE AllReduce of both payloads [38, 257]; staging and
        # return halves ride the two HWDGE rings in parallel (the bulk
        # has drained by then), gpsimd only triggers the collective ----
        b_in = DP.tile([2 * K, CCW], F32, name="b_in")
        b_out = DP.tile([2 * K, CCW], F32, name="b_out")
        for s in (0, 1):
            qs[s].dma_start(b_in[s * K:(s + 1) * K, :],
                            cc_in[32 * s:32 * s + K, :])
        cc = nc.gpsimd.collective_compute(
            "AllReduce", mybir.AluOpType.add,
            replica_groups=[list(range(n_cores))],
            ins=[b_in.opt()], outs=[b_out.opt()])
        bass._add_dep_helper(cc.ins, prev_cc.ins, sync=False,
                             reason="collectives in program order")
        for s in (0, 1):
            qs[s].dma_start(cs[32 * s:32 * s + K, :],
                            b_out[s * K:(s + 1) * K, :])

        # ---- Pearson prep, both sides at once ([51] partitions).
        # ms holds row SUMS; mean m = ms/C, and
        # sum_c (X-m)^2 = sum_c X^2 - ms^2/C  (no explicit centering) ----
        X = cs[:, 0:C]
        ms = cs[:, C:C + 1]
        xsq = PP.tile([32 + K, C], F32, name="xsq")
        ssx = PP.tile([32 + K, 1], F32, name="ssx")
        nc.scalar.activation(xsq[:], X,
                             mybir.ActivationFunctionType.Square,
                             accum_out=ssx[:])
        msc = PP.tile([32 + K, 1], F32, name="msc")
        nc.vector.tensor_scalar_mul(msc[:], ms, 1.0 / C)
        w = PP.tile([32 + K, 1], F32, name="w")
        nc.vector.tensor_scalar(w[:], ms, ms, 1.0 / C,
                                op0=mult, op1=mult)
        ss = PP.tile([32 + K, 1], F32, name="ss")
        nc.vector.tensor_sub(ss[:], ssx[:], w[:])
        sd = PP.tile([32 + K, 1], F32, name="sd")
        nc.scalar.sqrt(sd[:], ss[:])
        ri = PP.tile([32 + K, 1], F32, name="ri")
        nc.vector.reciprocal(ri[:], sd[:])
        # split per side so the transpose's lhsT sits at partition base
        # 0 / 32 (PE requires base in {0, 32, 64})
        xn = PP.tile([32 + K, C], F32, name="xn")
        for s in (0, 1):
            nc.vector.tensor_scalar(xn[32 * s:32 * s + K, :],
                                    X[32 * s:32 * s + K, :],
                                    msc[32 * s:32 * s + K, :],
                                    ri[32 * s:32 * s + K, :],
                                    op0=mybir.AluOpType.subtract,
                                    op1=mybir.AluOpType.mult)
        # transpose both sides' [K, C] -> [C, K], 4 blocks of 128
        tps = TLP.tile([128, 4 * K], F32, name="tps", tag="tlp")
        for j in range(4):
            s, h = j // 2, j % 2
            nc.tensor.matmul(
                tps[:, j * K:(j + 1) * K],
                lhsT=xn[32 * s:32 * s + K, h * 128:(h + 1) * 128],
                rhs=id_f[32 * s:32 * s + K, :],
                is_transpose=True,
                start=(j == 0), stop=(j == 3))
        nT = PP.tile([128, 4 * K], F32, name="nT")
        nc.vector.tensor_copy(nT[:], tps[:])

        # ---- final correlation ----
        po = TLP.tile([K, K], F32, name="po", tag="tlp")
        for h in (0, 1):
            nc.tensor.matmul(po[:],
                             lhsT=nT[:, h * K:(h + 1) * K],
                             rhs=nT[:, (2 + h) * K:(3 + h) * K],
                             start=(h == 0), stop=(h == 1))
        osb = PP.tile([K, K], F32, name="osb")
        nc.vector.tensor_copy(osb[:], po[:])
        nc.sync.dma_start(out_d[:], osb[:])


def build(n_cores=N_CORES):
    nc = bacc.Bacc("TRN2", target_bir_lowering=False, debug=False,
                   enable_asserts=False, num_devices=n_cores)
    pret_d = [nc.dram_tensor(f"pret{s}", [128, NCHUNK * K], BF16,
                             kind="ExternalInput").ap() for s in (1, 2)]
    ftr_d = [nc.dram_tensor(f"ftr{s}", [128, NCHUNK * CCW], BF16,
                            kind="ExternalInput").ap() for s in (1, 2)]
    identf_d = nc.dram_tensor("identf", [K, K], F32, kind="ExternalInput").ap()
    out_d = nc.dram_tensor("out", [K, K], F32, kind="ExternalOutput").ap()
    with tile.TileContext(nc) as tc:
        build_body(nc, tc, pret_d, ftr_d, identf_d, out_d, n_cores)
    nc.compile()
    return nc


_NC_CACHE = {}


def _get_nc():
    if "nc" not in _NC_CACHE:
        _NC_CACHE["nc"] = build(N_CORES)
    return _NC_CACHE["nc"]


class Runner:
    """Executes the compiled Bass program on the first `n_cores` jax
    devices via shard_map, with inputs pre-staged on the devices (the
    analog of the native path's input pre-load in run_neff) so all
    cores start the NEFF near-simultaneously."""

    def __init__(self, nc, n_cores):
        import jax
        from jax.experimental.shard_map import shard_map
        from jax.sharding import Mesh, PartitionSpec, NamedSharding

        bass2jax.install_neuronx_cc_hook()
        self.jax = jax
        self.nc = nc
        self.n_cores = n_cores
        assert nc.dbg_addr is None
        partition_name = (nc.partition_id_tensor.name
                          if nc.partition_id_tensor else None)
        in_names, out_names, out_avals = [], [], []
        for alloc in nc.m.functions[0].allocations:
            if not isinstance(alloc, mybir.MemoryLocationSet):
                continue
            name = alloc.memorylocations[0].name
            if alloc.kind == "ExternalInput":
                if name != partition_name:
                    in_names.append(name)
            elif alloc.kind == "ExternalOutput":
                shape = tuple(alloc.tensor_shape)
                dtype = mybir.dt.np(alloc.dtype)
                out_names.append(name)
                out_avals.append(jax.core.ShapedArray(shape, dtype))
        self.param_names = list(in_names)
        n_params = len(in_names)
        full_in_names = list(in_names) + list(out_names)
        if partition_name is not None:
            full_in_names.append(partition_name)
        full_in_names = tuple(full_in_names)
        donate = tuple(range(n_params, n_params + len(out_names)))
        self.out_names = out_names
        self.out_avals = out_avals

        def _body(*args):
            operands = list(args)
            if partition_name is not None:
                operands.append(bass2jax.partition_id_tensor())
            outs = bass2jax._bass_exec_p.bind(
                *operands,
                out_avals=tuple(out_avals),
                in_names=full_in_names,
                out_names=tuple(out_names),
                lowering_input_output_aliases=(),
                sim_require_finite=True,
                sim_require_nnan=True,
                nc=nc,
            )
            return tuple(outs)

        devices = jax.devices()[:n_cores]
        assert len(devices) == n_cores
        self.mesh = Mesh(np.asarray(devices), ("core",))
        in_specs = (PartitionSpec("core"),) * (n_params + len(out_names))
        out_specs = (PartitionSpec("core"),) * len(out_names)
        self.fn = jax.jit(
            shard_map(_body, mesh=self.mesh, in_specs=in_specs,
                      out_specs=out_specs, check_rep=False),
            donate_argnums=donate, keep_unused=True)
        self.sharding = NamedSharding(self.mesh, PartitionSpec("core"))

    def put(self, in_maps):
        concat = [
            np.concatenate([np.asarray(in_maps[c][n])
                            for c in range(self.n_cores)], axis=0)
            for n in self.param_names
        ]
        arrs = [self.jax.device_put(a, self.sharding) for a in concat]
        self.jax.block_until_ready(arrs)
        return arrs

    def zeros(self):
        zs = [self.jax.device_put(
            np.zeros((self.n_cores * a.shape[0], *a.shape[1:]), a.dtype),
            self.sharding) for a in self.out_avals]
        self.jax.block_until_ready(zs)
        return zs

    def exec(self, dev_in):
        outs = self.fn(*dev_in, *self.zeros())
        self.jax.block_until_ready(outs)
        return {
            name: np.asarray(outs[i]).reshape(
                self.n_cores, *self.out_avals[i].shape)
            for i, name in enumerate(self.out_names)
        }


def _get_runner():
    if "runner" not in _NC_CACHE:
        _NC_CACHE["runner"] = Runner(_get_nc(), N_CORES)
    return _NC_CACHE["runner"]


def make_in_maps(preds1, feats1, preds2, feats2):
    import ml_dtypes
    bf16 = ml_dtypes.bfloat16
    identf = np.eye(K, dtype=np.float32)
    per_stream = {}
    for s, (preds, feats) in enumerate(
            ((preds1, feats1), (preds2, feats2)), start=1):
        # preds [B,K,H,W] -> [B, W(v), H(u), K] -> [B, 128, 128*19]:
        # chunk u's columns are P^T[u*128:(u+1)*128, :19] with the
        # spatial index on partitions
        pr = np.ascontiguousarray(
            preds.astype(bf16).transpose(0, 3, 2, 1)
        ).reshape(B, 128, NCHUNK * K)
        # feats [B,C,H,W] -> [B, W, H, C (+ ones)] -> [B, 128, 128*257]:
        # chunk u is the [w, c] block at h=u, matching pret's chunking;
        # the fused ones column makes psum[:, 256] the mask sums
        ft = np.empty((B, W, H, CCW), dtype=bf16)
        ft[..., :C] = feats.astype(bf16).transpose(0, 3, 2, 1)
        ft[..., C] = 1.0
        per_stream[s] = (pr, ft.reshape(B, 128, NCHUNK * CCW))
    in_maps = []
    for b in range(B):
        in_maps.append({
            "pret1": per_stream[1][0][b],
            "pret2": per_stream[2][0][b],
            "ftr1": per_stream[1][1][b],
            "ftr2": per_stream[2][1][b],
            "identf": identf,
        })
    return in_maps


def kernel(preds1, feats1, preds2, feats2):
    runner = _get_runner()
    in_maps = make_in_maps(preds1, feats1, preds2, feats2)
    dev_in = runner.put(in_maps)
    outs = runner.exec(dev_in)
    return np.asarray(outs["out"][0], dtype=np.float32)


# revision 22
# speedup vs baseline: 1.0645x; 1.0235x over previous
"""Trainium2 Bass kernel for nn_CategoryAlign_Module (pooling / cross Pearson).

Math (see reference):
  for each stream s in {1,2}:
    vec_b[k,c]  = sum_p preds[b,k,p] * feats[b,c,p] / sum_p preds[b,k,p]
    ctx_b[k,c]  = vec_b[k,c] / max(||vec_b[:,c]||_2, 1e-12)      (norm over K)
    ctx[k,c]    = mean_b ctx_b[k,c]
  out = pearson(ctx1, ctx2)   (center+normalize rows over C, then ctx1 @ ctx2^T)

Distribution: data-parallel over the batch dim, one batch element per
NeuronCore (B=8, 8 cores).  Each core computes its local normalized
contexts, the tiny [19,257] payload is AllReduce-summed across the 8
cores (Pearson is invariant to the 1/B scale, so the mean's division is
skipped), and every core redundantly computes the replicated [19,19]
correlation.

Per-core pipeline (bf16 compute / fp32 accumulate):
  - both preds and feats are host-relayouted and host-cast to bf16, so
    the device only moves half the bytes and never transposes:
      preds -> [128, 128*19]  (chunk h: P^T[h*128:(h+1)*128, :19])
      feats -> [128, 128*257] (chunk h: [w, c] block with a fused ones
               column, so psum[:, 256] accumulates the mask sums)
  - bulk DMA is split across the two HWDGE queues (sync + scalar
    engines), 16-chunk segments, so both queues stream concurrently
  - 128 accumulating matmuls per stream produce [19, 257] in PSUM
  - stream 0's AllReduce launches at the halfway point and hides under
    stream 1's DMA; only stream 1's AllReduce + the tiny Pearson
    epilogue are exposed at the tail
"""

import sys

sys.path.insert(0, "/opt/trn_rl_repo")

import numpy as np

import concourse.bass as bass  # noqa: F401  (import order matters)
import concourse.bacc as bacc
import concourse.tile as tile
import concourse.mybir as mybir
from concourse import bass_utils, bass2jax  # noqa: F401

B, K, C, H, W = 8, 19, 256, 128, 128
P = H * W            # 16384 spatial positions
NCHUNK = P // 128    # 128 contraction chunks
CCW = C + 1          # channels + fused ones column (mask sums)
# DMA segment sizes in 128-position chunks: big segments for the bulk
# of the window, small ones at the end so the last arrival gates as
# little compute as possible
SEG_SIZES = (32, 32, 32, 16, 8, 8)
N_CORES = 8

F32 = mybir.dt.float32
BF16 = mybir.dt.bfloat16


def build_body(nc, tc, pret_d, ftrseg_d, identf_d, out_d, n_cores):
    """Emit the per-core program.

    pret_d: 2 DRAM APs [128, NCHUNK*K] bf16 (preds, spatial-major relayout)
    ftr_d:  2 DRAM APs [128, NCHUNK*CCW] bf16 (feats, spatial-major
            relayout + ones column)
    identf_d: [K, K] fp32 identity (for the tiny Pearson transposes)
    out_d:  [K, K] fp32 output
    """
    add = mybir.AluOpType.add
    mult = mybir.AluOpType.mult

    with tc.tile_pool(name="persist", bufs=1) as PP, \
         tc.tile_pool(name="ft", bufs=1) as FTP, \
         tc.tile_pool(name="acc", bufs=1, space="PSUM") as PA, \
         tc.tile_pool(name="tail", bufs=1, space="PSUM") as TLP, \
         tc.tile_pool(name="dram", bufs=1, space="DRAM") as DP:

        ones19 = PP.tile([K, 1], F32, name="ones19")
        nc.vector.memset(ones19[:], 1.0)
        onesrow = PP.tile([1, K], F32, name="onesrow")
        nc.vector.memset(onesrow[:], 1.0)

        # --- bulk DMA: both streams across the two HWDGE queues,
        # interleaved so stream 0 lands in the first half of the DMA
        # window.  The tiny constant loads and the warmup-collective
        # staging ride the same rings right after the first segments. ---
        qs = [nc.sync, nc.scalar]
        id_f = PP.tile([32 + K, K], F32, name="id_f")
        dw_in = DP.tile([1, 1], F32, name="dw_in")
        dw_out = DP.tile([1, 1], F32, name="dw_out")
        PT = []
        for s in (0, 1):
            pt = PP.tile([128, NCHUNK * K], BF16, name=f"PT{s}")
            qs[s].dma_start(pt[:], pret_d[s][:])
            PT.append(pt)
        fseg = [[], []]   # list of (tile, chunk_base, n_chunks)
        for s in (0, 1):
            base = 0
            for si, nch in enumerate(SEG_SIZES):
                t_ = FTP.tile([128, nch * CCW], BF16, name=f"fs{s}{si}")
                # each segment is its own exactly-sized DRAM tensor, so
                # every bulk transfer is one fully contiguous block read
                qs[(si + s) % 2].dma_start(t_[:], ftrseg_d[s][si][:])
                fseg[s].append((t_, base, nch))
                base += nch
                if s == 0 and si == 0:
                    nc.sync.dma_start(dw_in[:], ones19[0:1, 0:1])
                    nc.sync.dma_start(id_f[0:K, :], identf_d[:])
                elif s == 0 and si == 1:
                    nc.scalar.dma_start(id_f[32:32 + K, :], identf_d[:])

        # --- warmup AllReduce: absorbs the collective stack's one-time
        # trigger/mesh startup (~11us) while the bulk DMA streams; only
        # the gpsimd engine blocks on it. ---
        prev_cc = nc.gpsimd.collective_compute(
            "AllReduce", mybir.AluOpType.add,
            replica_groups=[list(range(n_cores))],
            ins=[dw_in.opt()], outs=[dw_out.opt()])

        # --- dummy activations preload the scalar engine's Square /
        # Rsqrt / Copy tables so the epilogues don't stall on
        # ACT_TABLE_LOAD (hidden under the bulk DMA window) ---
        wa = PP.tile([1, 1], F32, name="wa")
        wb = PP.tile([1, 1], F32, name="wb")
        nc.scalar.square(wa[:], ones19[0:1, 0:1])
        nc.scalar.sqrt(wb[:], ones19[0:1, 0:1])
        nc.scalar.activation(wa[:], ones19[0:1, 0:1],
                             mybir.ActivationFunctionType.Copy,
                             scale=0.5, accum_out=wb[:])

        # --- per-stream accumulators + shared payload tile (side s at
        # partition base 32*s: engines need operand bases in {0,32,64}) ---
        psum_vec = [PA.tile([K, CCW], F32, name=f"pvec{s}") for s in (0, 1)]
        cc_in = PP.tile([32 + K, CCW], F32, name="cc_in")
        cs = PP.tile([32 + K, CCW], F32, name="csum")
        # pad rows 19..31 so the merged tail ops stay finite there
        nc.vector.memset(cs[0:32, 0:C], 1.0)
        nc.vector.memset(cs[0:32, C:CCW], 0.0)

        for s in (0, 1):
            # ---- contraction: 128 accumulating matmuls ----
            for t_, base, nch in fseg[s]:
                for t in range(nch):
                    i = base + t
                    nc.tensor.matmul(
                        psum_vec[s][:],
                        lhsT=PT[s][:, i * K:(i + 1) * K],
                        rhs=t_[:, t * CCW:(t + 1) * CCW],
                        start=(i == 0), stop=(i == NCHUNK - 1))

            # ---- stream epilogue (stream 0's is fully hidden; only
            # stream 1's chain precedes the collective) ----
            # vec = psum[:, :C] / masksum;  ctx = vec / ||vec||_col
            rows = cc_in[32 * s:32 * s + K, :]
            recip = PP.tile([K, 1], F32, name="recip")
            nc.vector.reciprocal(recip[:], psum_vec[s][:, C:C + 1])
            rsq = PP.tile([K, 1], F32, name="rsq")
            nc.vector.tensor_mul(rsq[:], recip[:], recip[:])
            sq = PP.tile([K, C], F32, name="sq")
            nc.scalar.square(sq[:], psum_vec[s][:, 0:C])
            # col sums over K of (psum^2 * recip^2) = ||vec||^2
            pn = TLP.tile([1, C], F32, name="pn", tag="tlp")
            nc.tensor.matmul(pn[:], lhsT=rsq[:], rhs=sq[:],
                             start=True, stop=True)
            # reference clamps the norm at 1e-12; the norm here is
            # O(1e-2) for non-degenerate input, so the clamp is a no-op.
            nsb = PP.tile([1, C], F32, name="nsb")
            nc.scalar.sqrt(nsb[:], pn[:])
            rn = PP.tile([1, C], F32, name="rn")
            nc.vector.reciprocal(rn[:], nsb[:])
            # broadcast 1/norm to the K partitions (rank-1 matmul)
            bc = TLP.tile([K, C], F32, name="bc", tag="tlp")
            nc.tensor.matmul(bc[:], lhsT=onesrow[:], rhs=rn[:],
                             start=True, stop=True)
            bcs = PP.tile([K, C], F32, name="bcs")
            nc.scalar.copy(bcs[:], bc[:])
            # fused: ctx = (psum * recip) * bc, with the row-SUM shipped
            # in the payload's last column (the tail divides it by C)
            nc.vector.scalar_tensor_tensor(
                rows[:, 0:C], psum_vec[s][:, 0:C], recip[:], bcs[:],
                op0=mult, op1=mult, accum_out=rows[:, C:C + 1])

        # ---- ONE AllReduce of both payloads [38, 257]; staging and
        # return halves ride the two HWDGE rings in parallel (the bulk
        # has drained by then), gpsimd only triggers the collective ----
        b_in = DP.tile([2 * K, CCW], F32, name="b_in")
        b_out = DP.tile([2 * K, CCW], F32, name="b_out")
        for s in (0, 1):
            qs[s].dma_start(b_in[s * K:(s + 1) * K, :],
                            cc_in[32 * s:32 * s + K, :])
        cc = nc.gpsimd.collective_compute(
            "AllReduce", mybir.AluOpType.add,
            replica_groups=[list(range(n_cores))],
            ins=[b_in.opt()], outs=[b_out.opt()])
        bass._add_dep_helper(cc.ins, prev_cc.ins, sync=False,
                             reason="collectives in program order")
        for s in (0, 1):
            qs[s].dma_start(cs[32 * s:32 * s + K, :],
                            b_out[s * K:(s + 1) * K, :])

        # ---- Pearson prep, both sides at once ([51] partitions).
        # ms holds row SUMS; mean m = ms/C, and
        # sum_c (X-m)^2 = sum_c X^2 - ms^2/C  (no explicit centering) ----
        X = cs[:, 0:C]
        ms = cs[:, C:C + 1]
        xsq = PP.tile([32 + K, C], F32, name="xsq")
        ssx = PP.tile([32 + K, 1], F32, name="ssx")
        nc.scalar.activation(xsq[:], X,
                             mybir.ActivationFunctionType.Square,
                             accum_out=ssx[:])
        msc = PP.tile([32 + K, 1], F32, name="msc")
        nc.vector.tensor_scalar_mul(msc[:], ms, 1.0 / C)
        w = PP.tile([32 + K, 1], F32, name="w")
        nc.vector.tensor_scalar(w[:], ms, ms, 1.0 / C,
                                op0=mult, op1=mult)
        ss = PP.tile([32 + K, 1], F32, name="ss")
        nc.vector.tensor_sub(ss[:], ssx[:], w[:])
        sd = PP.tile([32 + K, 1], F32, name="sd")
        nc.scalar.sqrt(sd[:], ss[:])
        ri = PP.tile([32 + K, 1], F32, name="ri")
        nc.vector.reciprocal(ri[:], sd[:])
        # split per side so the transpose's lhsT sits at partition base
        # 0 / 32 (PE requires base in {0, 32, 64})
        xn = PP.tile([32 + K, C], F32, name="xn")
        for s in (0, 1):
            nc.vector.tensor_scalar(xn[32 * s:32 * s + K, :],
                                    X[32 * s:32 * s + K, :],
                                    msc[32 * s:32 * s + K, :],
                                    ri[32 * s:32 * s + K, :],
                                    op0=mybir.AluOpType.subtract,
                                    op1=mybir.AluOpType.mult)
        # transpose both sides' [K, C] -> [C, K], 4 blocks of 128
        tps = TLP.tile([128, 4 * K], F32, name="tps", tag="tlp")
        for j in range(4):
            s, h = j // 2, j % 2
            nc.tensor.matmul(
                tps[:, j * K:(j + 1) * K],
                lhsT=xn[32 * s:32 * s + K, h * 128:(h + 1) * 128],
                rhs=id_f[32 * s:32 * s + K, :],
                is_transpose=True,
                start=(j == 0), stop=(j == 3))
        nT = PP.tile([128, 4 * K], F32, name="nT")
        nc.vector.tensor_copy(nT[:], tps[:])

        # ---- final correlation ----
        po = TLP.tile([K, K], F32, name="po", tag="tlp")
        for h in (0, 1):
            nc.tensor.matmul(po[:],
                             lhsT=nT[:, h * K:(h + 1) * K],
                             rhs=nT[:, (2 + h) * K:(3 + h) * K],
                             start=(h == 0), stop=(h == 1))
        osb = PP.tile([K, K], F32, name="osb")
        nc.vector.tensor_copy(osb[:], po[:])
        nc.sync.dma_start(out_d[:], osb[:])


def build(n_cores=N_CORES):
    nc = bacc.Bacc("TRN2", target_bir_lowering=False, debug=False,
                   enable_asserts=False, num_devices=n_cores)
    pret_d = [nc.dram_tensor(f"pret{s}", [128, NCHUNK * K], BF16,
                             kind="ExternalInput").ap() for s in (1, 2)]
    ftrseg_d = [
        [nc.dram_tensor(f"ftr{s + 1}s{si}", [128, nch * CCW], BF16,
                        kind="ExternalInput").ap()
         for si, nch in enumerate(SEG_SIZES)]
        for s in (0, 1)]
    identf_d = nc.dram_tensor("identf", [K, K], F32, kind="ExternalInput").ap()
    out_d = nc.dram_tensor("out", [K, K], F32, kind="ExternalOutput").ap()
    with tile.TileContext(nc) as tc:
        build_body(nc, tc, pret_d, ftrseg_d, identf_d, out_d, n_cores)
    nc.compile()
    return nc


_NC_CACHE = {}


def _get_nc():
    if "nc" not in _NC_CACHE:
        _NC_CACHE["nc"] = build(N_CORES)
    return _NC_CACHE["nc"]


class Runner:
    """Executes the compiled Bass program on the first `n_cores` jax
    devices via shard_map, with inputs pre-staged on the devices (the
    analog of the native path's input pre-load in run_neff) so all
    cores start the NEFF near-simultaneously."""

    def __init__(self, nc, n_cores):
        import jax
        from jax.experimental.shard_map import shard_map
        from jax.sharding import Mesh, PartitionSpec, NamedSharding

        bass2jax.install_neuronx_cc_hook()
        self.jax = jax
        self.nc = nc
        self.n_cores = n_cores
        assert nc.dbg_addr is None
        partition_name = (nc.partition_id_tensor.name
                          if nc.partition_id_tensor else None)
        in_names, out_names, out_avals = [], [], []
        for alloc in nc.m.functions[0].allocations:
            if not isinstance(alloc, mybir.MemoryLocationSet):
                continue
            name = alloc.memorylocations[0].name
            if alloc.kind == "ExternalInput":
                if name != partition_name:
                    in_names.append(name)
            elif alloc.kind == "ExternalOutput":
                shape = tuple(alloc.tensor_shape)
                dtype = mybir.dt.np(alloc.dtype)
                out_names.append(name)
                out_avals.append(jax.core.ShapedArray(shape, dtype))
        self.param_names = list(in_names)
        n_params = len(in_names)
        full_in_names = list(in_names) + list(out_names)
        if partition_name is not None:
            full_in_names.append(partition_name)
        full_in_names = tuple(full_in_names)
        donate = tuple(range(n_params, n_params + len(out_names)))
        self.out_names = out_names
        self.out_avals = out_avals

        def _body(*args):
            operands = list(args)
            if partition_name is not None:
                operands.append(bass2jax.partition_id_tensor())
            outs = bass2jax._bass_exec_p.bind(
                *operands,
                out_avals=tuple(out_avals),
                in_names=full_in_names,
                out_names=tuple(out_names),
                lowering_input_output_aliases=(),
                sim_require_finite=True,
                sim_require_nnan=True,
                nc=nc,
            )
            return tuple(outs)

        devices = jax.devices()[:n_cores]
        assert len(devices) == n_cores
        self.mesh = Mesh(np.asarray(devices), ("core",))
        in_specs = (PartitionSpec("core"),) * (n_params + len(out_names))
        out_specs = (PartitionSpec("core"),) * len(out_names)
        self.fn = jax.jit(
            shard_map(_body, mesh=self.mesh, in_specs=in_specs,
                      out_specs=out_specs, check_rep=False),
            donate_argnums=donate, keep_unused=True)
        self.sharding = NamedSharding(self.mesh, PartitionSpec("core"))

    def put(self, in_maps):
        concat = [
            np.concatenate([np.asarray(in_maps[c][n])
                            for c in range(self.n_cores)], axis=0)
            for n in self.param_names
        ]
        arrs = [self.jax.device_put(a, self.sharding) for a in concat]
        self.jax.block_until_ready(arrs)
        return arrs

    def zeros(self):
        zs = [self.jax.device_put(
            np.zeros((self.n_cores * a.shape[0], *a.shape[1:]), a.dtype),
            self.sharding) for a in self.out_avals]
        self.jax.block_until_ready(zs)
        return zs

    def exec(self, dev_in):
        outs = self.fn(*dev_in, *self.zeros())
        self.jax.block_until_ready(outs)
        return {
            name: np.asarray(outs[i]).reshape(
                self.n_cores, *self.out_avals[i].shape)
            for i, name in enumerate(self.out_names)
        }


def _get_runner():
    if "runner" not in _NC_CACHE:
        _NC_CACHE["runner"] = Runner(_get_nc(), N_CORES)
    return _NC_CACHE["runner"]


def make_in_maps(preds1, feats1, preds2, feats2):
    import ml_dtypes
    bf16 = ml_dtypes.bfloat16
    identf = np.eye(K, dtype=np.float32)
    per_stream = {}
    for s, (preds, feats) in enumerate(
            ((preds1, feats1), (preds2, feats2)), start=1):
        # preds [B,K,H,W] -> [B, W(v), H(u), K] -> [B, 128, 128*19]:
        # chunk u's columns are P^T[u*128:(u+1)*128, :19] with the
        # spatial index on partitions
        pr = np.ascontiguousarray(
            preds.astype(bf16).transpose(0, 3, 2, 1)
        ).reshape(B, 128, NCHUNK * K)
        # feats [B,C,H,W] -> [B, W, H, C (+ ones)] -> [B, 128, 128*257]:
        # chunk u is the [w, c] block at h=u, matching pret's chunking;
        # the fused ones column makes psum[:, 256] the mask sums
        ft = np.empty((B, W, H, CCW), dtype=bf16)
        ft[..., :C] = feats.astype(bf16).transpose(0, 3, 2, 1)
        ft[..., C] = 1.0
        per_stream[s] = (pr, ft.reshape(B, 128, NCHUNK * CCW))
    in_maps = []
    for b in range(B):
        m = {
            "pret1": per_stream[1][0][b],
            "pret2": per_stream[2][0][b],
            "identf": identf,
        }
        for s in (1, 2):
            base = 0
            for si, nch in enumerate(SEG_SIZES):
                m[f"ftr{s}s{si}"] = np.ascontiguousarray(
                    per_stream[s][1][b][:, base * CCW:(base + nch) * CCW])
                base += nch
        in_maps.append(m)
    return in_maps


def kernel(preds1, feats1, preds2, feats2):
    runner = _get_runner()
    in_maps = make_in_maps(preds1, feats1, preds2, feats2)
    dev_in = runner.put(in_maps)
    outs = runner.exec(dev_in)
    return np.asarray(outs["out"][0], dtype=np.float32)


# revision 23
# speedup vs baseline: 1.1596x; 1.0893x over previous
"""Trainium2 Bass kernel for nn_CategoryAlign_Module (pooling / cross Pearson).

Math (see reference):
  for each stream s in {1,2}:
    vec_b[k,c]  = sum_p preds[b,k,p] * feats[b,c,p] / sum_p preds[b,k,p]
    ctx_b[k,c]  = vec_b[k,c] / max(||vec_b[:,c]||_2, 1e-12)      (norm over K)
    ctx[k,c]    = mean_b ctx_b[k,c]
  out = pearson(ctx1, ctx2)   (center+normalize rows over C, then ctx1 @ ctx2^T)

Distribution: data-parallel over the batch dim, one batch element per
NeuronCore (B=8, 8 cores).  Each core computes its local normalized
contexts, the tiny [19,257] payload is AllReduce-summed across the 8
cores (Pearson is invariant to the 1/B scale, so the mean's division is
skipped), and every core redundantly computes the replicated [19,19]
correlation.

Per-core pipeline (bf16 compute / fp32 accumulate):
  - both preds and feats are host-relayouted and host-cast to bf16, so
    the device only moves half the bytes and never transposes:
      preds -> [128, 128*19]  (chunk h: P^T[h*128:(h+1)*128, :19])
      feats -> [128, 128*257] (chunk h: [w, c] block with a fused ones
               column, so psum[:, 256] accumulates the mask sums)
  - bulk DMA is split across the two HWDGE queues (sync + scalar
    engines), 16-chunk segments, so both queues stream concurrently
  - 128 accumulating matmuls per stream produce [19, 257] in PSUM
  - stream 0's AllReduce launches at the halfway point and hides under
    stream 1's DMA; only stream 1's AllReduce + the tiny Pearson
    epilogue are exposed at the tail
"""

import sys

sys.path.insert(0, "/opt/trn_rl_repo")

import numpy as np

import concourse.bass as bass  # noqa: F401  (import order matters)
import concourse.bacc as bacc
import concourse.tile as tile
import concourse.mybir as mybir
from concourse import bass_utils, bass2jax  # noqa: F401

B, K, C, H, W = 8, 19, 256, 128, 128
P = H * W            # 16384 spatial positions
NCHUNK = P // 128    # 128 contraction chunks
CCW = C + 1          # channels + fused ones column (mask sums)
# DMA segment sizes in 128-position chunks: big segments for the bulk
# of the window, small ones at the end so the last arrival gates as
# little compute as possible
SEG_SIZES = (32, 32, 32, 16, 12, 4)
N_CORES = 8

F32 = mybir.dt.float32
BF16 = mybir.dt.bfloat16


def build_body(nc, tc, pret_d, ftrseg_d, identf_d, out_d, n_cores):
    """Emit the per-core program.

    pret_d: 2 DRAM APs [128, NCHUNK*K] bf16 (preds, spatial-major relayout)
    ftr_d:  2 DRAM APs [128, NCHUNK*CCW] bf16 (feats, spatial-major
            relayout + ones column)
    identf_d: [K, K] fp32 identity (for the tiny Pearson transposes)
    out_d:  [K, K] fp32 output
    """
    add = mybir.AluOpType.add
    mult = mybir.AluOpType.mult

    with tc.tile_pool(name="persist", bufs=1) as PP, \
         tc.tile_pool(name="ft", bufs=1) as FTP, \
         tc.tile_pool(name="acc", bufs=1, space="PSUM") as PA, \
         tc.tile_pool(name="tail", bufs=1, space="PSUM") as TLP, \
         tc.tile_pool(name="dram", bufs=1, space="DRAM") as DP:

        ones19 = PP.tile([K, 1], F32, name="ones19")
        nc.vector.memset(ones19[:], 1.0)
        onesrow = PP.tile([1, K], F32, name="onesrow")
        nc.vector.memset(onesrow[:], 1.0)

        # --- bulk DMA: both streams across the two HWDGE queues,
        # interleaved so stream 0 lands in the first half of the DMA
        # window.  The tiny constant loads and the warmup-collective
        # staging ride the same rings right after the first segments. ---
        qs = [nc.sync, nc.scalar]
        id_f = PP.tile([32 + K, K], F32, name="id_f")
        dw_in = DP.tile([1, 1], F32, name="dw_in")
        dw_out = DP.tile([1, 1], F32, name="dw_out")
        PT = []
        for s in (0, 1):
            pt = PP.tile([128, NCHUNK * K], BF16, name=f"PT{s}")
            qs[s].dma_start(pt[:], pret_d[s][:])
            PT.append(pt)
        fseg = [[], []]   # list of (tile, chunk_base, n_chunks)
        for s in (0, 1):
            base = 0
            for si, nch in enumerate(SEG_SIZES):
                t_ = FTP.tile([128, nch * CCW], BF16, name=f"fs{s}{si}")
                # each segment is its own exactly-sized DRAM tensor, so
                # every bulk transfer is one fully contiguous block read
                qs[(si + s) % 2].dma_start(t_[:], ftrseg_d[s][si][:])
                fseg[s].append((t_, base, nch))
                base += nch
                if s == 0 and si == 0:
                    nc.sync.dma_start(dw_in[:], ones19[0:1, 0:1])
                    nc.sync.dma_start(id_f[0:K, :], identf_d[:])
                elif s == 0 and si == 1:
                    nc.scalar.dma_start(id_f[32:32 + K, :], identf_d[:])

        # --- warmup AllReduce: absorbs the collective stack's one-time
        # trigger/mesh startup (~11us) while the bulk DMA streams; only
        # the gpsimd engine blocks on it. ---
        prev_cc = nc.gpsimd.collective_compute(
            "AllReduce", mybir.AluOpType.add,
            replica_groups=[list(range(n_cores))],
            ins=[dw_in.opt()], outs=[dw_out.opt()])

        # --- dummy activations preload the scalar engine's Square /
        # Rsqrt / Copy tables so the epilogues don't stall on
        # ACT_TABLE_LOAD (hidden under the bulk DMA window) ---
        wa = PP.tile([1, 1], F32, name="wa")
        wb = PP.tile([1, 1], F32, name="wb")
        nc.scalar.square(wa[:], ones19[0:1, 0:1])
        nc.scalar.sqrt(wb[:], ones19[0:1, 0:1])
        nc.scalar.activation(wa[:], ones19[0:1, 0:1],
                             mybir.ActivationFunctionType.Copy,
                             scale=0.5, accum_out=wb[:])

        # --- per-stream accumulators + shared payload tile (side s at
        # partition base 32*s: engines need operand bases in {0,32,64}) ---
        psum_vec = [PA.tile([K, CCW], F32, name=f"pvec{s}") for s in (0, 1)]
        cc_in = PP.tile([32 + K, CCW], F32, name="cc_in")
        cs = PP.tile([32 + K, CCW], F32, name="csum")
        # pad rows 19..31 so the merged tail ops stay finite there
        nc.vector.memset(cs[0:32, 0:C], 1.0)
        nc.vector.memset(cs[0:32, C:CCW], 0.0)

        for s in (0, 1):
            # ---- contraction: 128 accumulating matmuls ----
            for t_, base, nch in fseg[s]:
                for t in range(nch):
                    i = base + t
                    nc.tensor.matmul(
                        psum_vec[s][:],
                        lhsT=PT[s][:, i * K:(i + 1) * K],
                        rhs=t_[:, t * CCW:(t + 1) * CCW],
                        start=(i == 0), stop=(i == NCHUNK - 1))

            # ---- stream epilogue (stream 0's is fully hidden; only
            # stream 1's chain precedes the collective) ----
            # vec = psum[:, :C] / masksum;  ctx = vec / ||vec||_col
            rows = cc_in[32 * s:32 * s + K, :]
            recip = PP.tile([K, 1], F32, name="recip")
            nc.vector.reciprocal(recip[:], psum_vec[s][:, C:C + 1])
            rsq = PP.tile([K, 1], F32, name="rsq")
            nc.vector.tensor_mul(rsq[:], recip[:], recip[:])
            sq = PP.tile([K, C], F32, name="sq")
            nc.scalar.square(sq[:], psum_vec[s][:, 0:C])
            # col sums over K of (psum^2 * recip^2) = ||vec||^2
            pn = TLP.tile([1, C], F32, name="pn", tag="tlp")
            nc.tensor.matmul(pn[:], lhsT=rsq[:], rhs=sq[:],
                             start=True, stop=True)
            # reference clamps the norm at 1e-12; the norm here is
            # O(1e-2) for non-degenerate input, so the clamp is a no-op.
            nsb = PP.tile([1, C], F32, name="nsb")
            nc.scalar.sqrt(nsb[:], pn[:])
            rn = PP.tile([1, C], F32, name="rn")
            nc.vector.reciprocal(rn[:], nsb[:])
            # broadcast 1/norm to the K partitions (rank-1 matmul)
            bc = TLP.tile([K, C], F32, name="bc", tag="tlp")
            nc.tensor.matmul(bc[:], lhsT=onesrow[:], rhs=rn[:],
                             start=True, stop=True)
            bcs = PP.tile([K, C], F32, name="bcs")
            nc.scalar.copy(bcs[:], bc[:])
            # fused: ctx = (psum * recip) * bc, with the row-SUM shipped
            # in the payload's last column (the tail divides it by C)
            nc.vector.scalar_tensor_tensor(
                rows[:, 0:C], psum_vec[s][:, 0:C], recip[:], bcs[:],
                op0=mult, op1=mult, accum_out=rows[:, C:C + 1])

        # ---- ONE AllReduce of both payloads [38, 257]; staging and
        # return halves ride the two HWDGE rings in parallel (the bulk
        # has drained by then), gpsimd only triggers the collective ----
        b_in = DP.tile([2 * K, CCW], F32, name="b_in")
        b_out = DP.tile([2 * K, CCW], F32, name="b_out")
        for s in (0, 1):
            qs[s].dma_start(b_in[s * K:(s + 1) * K, :],
                            cc_in[32 * s:32 * s + K, :])
        cc = nc.gpsimd.collective_compute(
            "AllReduce", mybir.AluOpType.add,
            replica_groups=[list(range(n_cores))],
            ins=[b_in.opt()], outs=[b_out.opt()])
        bass._add_dep_helper(cc.ins, prev_cc.ins, sync=False,
                             reason="collectives in program order")
        for s in (0, 1):
            qs[s].dma_start(cs[32 * s:32 * s + K, :],
                            b_out[s * K:(s + 1) * K, :])

        # ---- Pearson prep, both sides at once ([51] partitions).
        # ms holds row SUMS; mean m = ms/C, and
        # sum_c (X-m)^2 = sum_c X^2 - ms^2/C  (no explicit centering) ----
        X = cs[:, 0:C]
        ms = cs[:, C:C + 1]
        xsq = PP.tile([32 + K, C], F32, name="xsq")
        ssx = PP.tile([32 + K, 1], F32, name="ssx")
        nc.scalar.activation(xsq[:], X,
                             mybir.ActivationFunctionType.Square,
                             accum_out=ssx[:])
        msc = PP.tile([32 + K, 1], F32, name="msc")
        nc.vector.tensor_scalar_mul(msc[:], ms, 1.0 / C)
        w = PP.tile([32 + K, 1], F32, name="w")
        nc.vector.tensor_scalar(w[:], ms, ms, 1.0 / C,
                                op0=mult, op1=mult)
        ss = PP.tile([32 + K, 1], F32, name="ss")
        nc.vector.tensor_sub(ss[:], ssx[:], w[:])
        sd = PP.tile([32 + K, 1], F32, name="sd")
        nc.scalar.sqrt(sd[:], ss[:])
        ri = PP.tile([32 + K, 1], F32, name="ri")
        nc.vector.reciprocal(ri[:], sd[:])
        # split per side so the transpose's lhsT sits at partition base
        # 0 / 32 (PE requires base in {0, 32, 64})
        xn = PP.tile([32 + K, C], F32, name="xn")
        for s in (0, 1):
            nc.vector.tensor_scalar(xn[32 * s:32 * s + K, :],
                                    X[32 * s:32 * s + K, :],
                                    msc[32 * s:32 * s + K, :],
                                    ri[32 * s:32 * s + K, :],
                                    op0=mybir.AluOpType.subtract,
                                    op1=mybir.AluOpType.mult)
        # transpose both sides' [K, C] -> [C, K], 4 blocks of 128
        tps = TLP.tile([128, 4 * K], F32, name="tps", tag="tlp")
        for j in range(4):
            s, h = j // 2, j % 2
            nc.tensor.matmul(
                tps[:, j * K:(j + 1) * K],
                lhsT=xn[32 * s:32 * s + K, h * 128:(h + 1) * 128],
                rhs=id_f[32 * s:32 * s + K, :],
                is_transpose=True,
                start=(j == 0), stop=(j == 3))
        nT = PP.tile([128, 4 * K], F32, name="nT")
        nc.vector.tensor_copy(nT[:], tps[:])

        # ---- final correlation ----
        po = TLP.tile([K, K], F32, name="po", tag="tlp")
        for h in (0, 1):
            nc.tensor.matmul(po[:],
                             lhsT=nT[:, h * K:(h + 1) * K],
                             rhs=nT[:, (2 + h) * K:(3 + h) * K],
                             start=(h == 0), stop=(h == 1))
        osb = PP.tile([K, K], F32, name="osb")
        nc.vector.tensor_copy(osb[:], po[:])
        nc.sync.dma_start(out_d[:], osb[:])


def build(n_cores=N_CORES):
    nc = bacc.Bacc("TRN2", target_bir_lowering=False, debug=False,
                   enable_asserts=False, num_devices=n_cores)
    pret_d = [nc.dram_tensor(f"pret{s}", [128, NCHUNK * K], BF16,
                             kind="ExternalInput").ap() for s in (1, 2)]
    ftrseg_d = [
        [nc.dram_tensor(f"ftr{s + 1}s{si}", [128, nch * CCW], BF16,
                        kind="ExternalInput").ap()
         for si, nch in enumerate(SEG_SIZES)]
        for s in (0, 1)]
    identf_d = nc.dram_tensor("identf", [K, K], F32, kind="ExternalInput").ap()
    out_d = nc.dram_tensor("out", [K, K], F32, kind="ExternalOutput").ap()
    with tile.TileContext(nc) as tc:
        build_body(nc, tc, pret_d, ftrseg_d, identf_d, out_d, n_cores)
    nc.compile()
    return nc


_NC_CACHE = {}


def _get_nc():
    if "nc" not in _NC_CACHE:
        _NC_CACHE["nc"] = build(N_CORES)
    return _NC_CACHE["nc"]


class Runner:
    """Executes the compiled Bass program on the first `n_cores` jax
    devices via shard_map, with inputs pre-staged on the devices (the
    analog of the native path's input pre-load in run_neff) so all
    cores start the NEFF near-simultaneously."""

    def __init__(self, nc, n_cores):
        import jax
        from jax.experimental.shard_map import shard_map
        from jax.sharding import Mesh, PartitionSpec, NamedSharding

        bass2jax.install_neuronx_cc_hook()
        self.jax = jax
        self.nc = nc
        self.n_cores = n_cores
        assert nc.dbg_addr is None
        partition_name = (nc.partition_id_tensor.name
                          if nc.partition_id_tensor else None)
        in_names, out_names, out_avals = [], [], []
        for alloc in nc.m.functions[0].allocations:
            if not isinstance(alloc, mybir.MemoryLocationSet):
                continue
            name = alloc.memorylocations[0].name
            if alloc.kind == "ExternalInput":
                if name != partition_name:
                    in_names.append(name)
            elif alloc.kind == "ExternalOutput":
                shape = tuple(alloc.tensor_shape)
                dtype = mybir.dt.np(alloc.dtype)
                out_names.append(name)
                out_avals.append(jax.core.ShapedArray(shape, dtype))
        self.param_names = list(in_names)
        n_params = len(in_names)
        full_in_names = list(in_names) + list(out_names)
        if partition_name is not None:
            full_in_names.append(partition_name)
        full_in_names = tuple(full_in_names)
        donate = tuple(range(n_params, n_params + len(out_names)))
        self.out_names = out_names
        self.out_avals = out_avals

        def _body(*args):
            operands = list(args)
            if partition_name is not None:
                operands.append(bass2jax.partition_id_tensor())
            outs = bass2jax._bass_exec_p.bind(
                *operands,
                out_avals=tuple(out_avals),
                in_names=full_in_names,
                out_names=tuple(out_names),
                lowering_input_output_aliases=(),
                sim_require_finite=True,
                sim_require_nnan=True,
                nc=nc,
            )
            return tuple(outs)

        devices = jax.devices()[:n_cores]
        assert len(devices) == n_cores
        self.mesh = Mesh(np.asarray(devices), ("core",))
        in_specs = (PartitionSpec("core"),) * (n_params + len(out_names))
        out_specs = (PartitionSpec("core"),) * len(out_names)
        self.fn = jax.jit(
            shard_map(_body, mesh=self.mesh, in_specs=in_specs,
                      out_specs=out_specs, check_rep=False),
            donate_argnums=donate, keep_unused=True)
        self.sharding = NamedSharding(self.mesh, PartitionSpec("core"))

    def put(self, in_maps):
        concat = [
            np.concatenate([np.asarray(in_maps[c][n])
                            for c in range(self.n_cores)], axis=0)
            for n in self.param_names
        ]
        arrs = [self.jax.device_put(a, self.sharding) for a in concat]
        self.jax.block_until_ready(arrs)
        return arrs

    def zeros(self):
        zs = [self.jax.device_put(
            np.zeros((self.n_cores * a.shape[0], *a.shape[1:]), a.dtype),
            self.sharding) for a in self.out_avals]
        self.jax.block_until_ready(zs)
        return zs

    def exec(self, dev_in):
        outs = self.fn(*dev_in, *self.zeros())
        self.jax.block_until_ready(outs)
        return {
            name: np.asarray(outs[i]).reshape(
                self.n_cores, *self.out_avals[i].shape)
            for i, name in enumerate(self.out_names)
        }


def _get_runner():
    if "runner" not in _NC_CACHE:
        _NC_CACHE["runner"] = Runner(_get_nc(), N_CORES)
    return _NC_CACHE["runner"]


def make_in_maps(preds1, feats1, preds2, feats2):
    import ml_dtypes
    bf16 = ml_dtypes.bfloat16
    identf = np.eye(K, dtype=np.float32)
    per_stream = {}
    for s, (preds, feats) in enumerate(
            ((preds1, feats1), (preds2, feats2)), start=1):
        # preds [B,K,H,W] -> [B, W(v), H(u), K] -> [B, 128, 128*19]:
        # chunk u's columns are P^T[u*128:(u+1)*128, :19] with the
        # spatial index on partitions
        pr = np.ascontiguousarray(
            preds.astype(bf16).transpose(0, 3, 2, 1)
        ).reshape(B, 128, NCHUNK * K)
        # feats [B,C,H,W] -> [B, W, H, C (+ ones)] -> [B, 128, 128*257]:
        # chunk u is the [w, c] block at h=u, matching pret's chunking;
        # the fused ones column makes psum[:, 256] the mask sums
        ft = np.empty((B, W, H, CCW), dtype=bf16)
        ft[..., :C] = feats.astype(bf16).transpose(0, 3, 2, 1)
        ft[..., C] = 1.0
        per_stream[s] = (pr, ft.reshape(B, 128, NCHUNK * CCW))
    in_maps = []
    for b in range(B):
        m = {
            "pret1": per_stream[1][0][b],
            "pret2": per_stream[2][0][b],
            "identf": identf,
        }
        for s in (1, 2):
            base = 0
            for si, nch in enumerate(SEG_SIZES):
                m[f"ftr{s}s{si}"] = np.ascontiguousarray(
                    per_stream[s][1][b][:, base * CCW:(base + nch) * CCW])
                base += nch
        in_maps.append(m)
    return in_maps


def kernel(preds1, feats1, preds2, feats2):
    runner = _get_runner()
    in_maps = make_in_maps(preds1, feats1, preds2, feats2)
    dev_in = runner.put(in_maps)
    outs = runner.exec(dev_in)
    return np.asarray(outs["out"][0], dtype=np.float32)
